# revision 16
# baseline (speedup 1.0000x reference)
"""Trainium2 Bass kernel for nn_ReconstructionHead (dense_mlp).

Computes, for x[B=256, T=513, D=512] (CLS token at t=512 dropped):
    h   = x[:, :512] @ W1.T + b1          # [256, 512, 512]
    h   = LayerNorm(h) * gamma + beta     # over last dim
    h   = relu(h)
    out[b, t] = h[b, t] @ Wout[t] + bout[t]   # [256, 512]

Sharding: data-parallel over batch across 8 NeuronCores (32 batches/core).
Weights are replicated. All input reshaping/transposition happens on the
host (numpy); the device sees clean strided layouts.

Per-core device program (fast path, gamma==1 / beta==0):
  - x is pre-transposed on host to xt[p, dc, m] with d = dc*128+p on SBUF
    partitions, m = b_local*512 + t  (16384 rows -> 128 tiles of 128 rows).
  - PE: per tile, a K=1 matmul seeds PSUM with b1 (broadcast), then 4
    accumulating 128x128x512 fp32 matmuls produce h1 = x @ W1.T + b1.
  - DVE bn_stats/bn_aggr give mean/var per row.
  - ACT computes u = relu(h1 - mu) straight out of PSUM (per-partition bias).
    Since rstd > 0, relu((h1-mu)*rstd) = rstd * relu(h1-mu), so the rstd
    multiply is deferred to the tiny per-column epilogue.
  - One scalar_tensor_tensor with accum_out computes s = sum_e u * Wout[t]
    (the per-column head), split between GPSIMD and DVE to balance engines.
  - Epilogue: out_col = s * rstd + bout, accumulated in a [128, 128] SBUF
    tile, PE-transposed once at the end for a contiguous output DMA.
"""

import os
import sys

import numpy as np

for _p in ("/root/.axon_site/_ro/trn_rl_repo", "/opt/trn_rl_repo"):
    if os.path.isdir(_p) and _p not in sys.path:
        sys.path.append(_p)

B = 256
T = 513
D = 512          # d_in == d_out
NCORES = 8
BL = B // NCORES          # 32 batches per core
M = BL * D                # 16384 rows per core
NT = M // 128             # 128 tiles per core
NG = NT // 4              # 32 groups (one group = 512 rows = one batch)
EPS = 1e-5

_programs = {}


def _matmul_dtype():
    """'bf16' (default), 'f32r', or 'f32' via KERNEL_DTYPE."""
    d = os.environ.get("KERNEL_DTYPE", "bf16")
    if os.environ.get("KERNEL_FP32_STRICT") == "1":
        return "f32"
    return d


def _build_program(apply_gamma_beta: bool):
    import concourse.bacc as bacc
    import concourse.tile as tile
    from concourse import mybir

    f32 = mybir.dt.float32
    dt_mode = _matmul_dtype()
    # bf16 matmuls stream at the same 1 cycle/row as float32r but get
    # FWL on LDWEIGHTS (4x faster weight loads that hide under the
    # previous matmul) and halve DMA + stage-2 vector-engine traffic.
    if dt_mode == "bf16":
        f32m = mybir.dt.bfloat16
    elif dt_mode == "f32r":
        f32m = mybir.dt.float32r
    else:
        f32m = f32
    # vector-side dtype for relu output / stage-2 operands
    vd = mybir.dt.bfloat16 if dt_mode == "bf16" else f32
    Alu = mybir.AluOpType
    Act = mybir.ActivationFunctionType

    nc = bacc.Bacc()
    xt = nc.dram_tensor("xt", [128, NG, 4, 512], f32m, kind="ExternalInput")
    w1t = nc.dram_tensor("w1t", [128, 4, D], f32m, kind="ExternalInput")
    # b1/128 replicated over 128 partitions: the bias seed is a K=128
    # matmul (ones.T @ b1rep) so its LDWEIGHTS pipelines exactly like the
    # main matmuls (a K=1 rank-1 seed costs two ~100ns boundary bubbles)
    b1 = nc.dram_tensor("b1", [128, D], f32m, kind="ExternalInput")
    wout = nc.dram_tensor("wout", [128, 4, D], vd, kind="ExternalInput")
    bout = nc.dram_tensor("bout", [128, 4], f32, kind="ExternalInput")
    if apply_gamma_beta:
        gammab = nc.dram_tensor("gammab", [128, D], f32, kind="ExternalInput")
        betab = nc.dram_tensor("betab", [128, D], f32, kind="ExternalInput")
    # out[p, c] = output for row m = c*128 + p (transposed on host)
    out = nc.dram_tensor("out", [128, 128], f32, kind="ExternalOutput")

    with tile.TileContext(nc) as tc:
        with (
            tc.tile_pool(name="singles", bufs=1) as singles,
            tc.tile_pool(name="xg", bufs=4) as xpool,
            tc.tile_pool(name="u", bufs=8) as upool,
            tc.tile_pool(name="junk", bufs=4) as jpool,
            tc.tile_pool(name="stats", bufs=12) as spool,
            tc.tile_pool(name="grp", bufs=4) as gpool,
            tc.tile_pool(name="psum", bufs=8, space="PSUM") as psum_pool,
        ):
            # ---- static tiles ----
            # ordered so the first matmul's dependencies land first
            b1_sb = singles.tile([128, D], f32m)
            nc.sync.dma_start(b1_sb, b1[:, :])
            w1t_sb = singles.tile([128, 4, D], f32m)
            nc.sync.dma_start(w1t_sb, w1t[:, :, :])

            def load_group(g):
                xg = xpool.tile([128, 4, 512], f32m, tag="xg")
                nc.sync.dma_start(xg, xt[:, g, :, :])
                return xg

            xg_next = load_group(0)

            wout_sb = singles.tile([128, 4, D], vd)
            nc.sync.dma_start(wout_sb, wout[:, :, :])
            bout_sb = singles.tile([128, 4], f32)
            nc.sync.dma_start(bout_sb, bout[:, :])
            if f32m is mybir.dt.float32r:
                ones_sb = singles.tile([128, 128], f32)
                nc.vector.memset(ones_sb, 1.0)
                ones_mm = ones_sb.bitcast(f32m)
            else:
                ones_mm = singles.tile([128, 128], f32m)
                nc.vector.memset(ones_mm, 1.0)
            eps_sb = singles.tile([128, 1], f32)
            nc.vector.memset(eps_sb, EPS)
            ocol = singles.tile([128, 128], f32)  # per-tile output columns
            if apply_gamma_beta:
                gamma_sb = singles.tile([128, D], f32)
                nc.sync.dma_start(gamma_sb, gammab[:, :])
                beta_sb = singles.tile([128, D], f32)
                nc.sync.dma_start(beta_sb, betab[:, :])

            for g in range(NG):
                xg = xg_next
                if g + 1 < NG:
                    xg_next = load_group(g + 1)

                mvg = gpool.tile([128, 4, 2], f32)   # (mean, var) per tile
                sg = gpool.tile([128, 4], f32)       # stage-2 raw sums

                for i in range(4):
                    c = g * 4 + i
                    P = psum_pool.tile([128, 512], f32)
                    # seed PSUM with b1 (rank-1 matmul), then accumulate x@W1T
                    nc.tensor.matmul(P, ones_mm, b1_sb, start=True, stop=False)
                    for dc in range(4):
                        nc.tensor.matmul(
                            P,
                            xg[:, dc, i * 128:(i + 1) * 128],
                            w1t_sb[:, dc, :],
                            start=False,
                            stop=(dc == 3),
                        )

                    st6 = spool.tile([128, 6], f32)
                    nc.vector.bn_stats(st6, P)
                    nc.vector.bn_aggr(mvg[:, i, :], st6)

                    if not apply_gamma_beta:
                        # Host negated W1T/b1, so P holds -h1 and bn_stats'
                        # mean is -mu: u = relu(-1*P + mean) = relu(h1 - mu).
                        # rstd multiplication is deferred to the epilogue.
                        u = upool.tile([128, 512], vd)
                        nc.scalar.activation(
                            u, P, Act.Relu, bias=mvg[:, i, 0:1], scale=-1.0
                        )
                    else:
                        # full path: n = (h1 - mu) * rstd ; z = n*gamma + beta
                        sd = spool.tile([128, 1], f32, tag="sd")
                        nc.scalar.activation(
                            sd, mvg[:, i, 1:2], Act.Sqrt, bias=eps_sb, scale=1.0
                        )
                        rr = spool.tile([128, 1], f32, tag="rr")
                        nc.vector.reciprocal(rr, sd)
                        n_sb = upool.tile([128, 512], f32, tag="n")
                        nc.vector.tensor_scalar(
                            out=n_sb,
                            in0=P,
                            scalar1=mvg[:, i, 0:1],
                            scalar2=rr,
                            op0=Alu.subtract,
                            op1=Alu.mult,
                        )
                        v_sb = upool.tile([128, 512], f32, tag="v")
                        nc.gpsimd.tensor_mul(v_sb, n_sb, gamma_sb)
                        z_sb = upool.tile([128, 512], f32, tag="z")
                        nc.vector.tensor_add(z_sb, v_sb, beta_sb)
                        u = upool.tile([128, 512], vd)
                        nc.scalar.activation(u, z_sb, Act.Relu)

                    # stage-2: s = sum_e u * Wout[t-block i]
                    # 1/3 on DVE (stt is its 3rd pass after bn_stats+aggr),
                    # 2/3 on GPSIMD+ACT which have more headroom
                    junk = jpool.tile([128, 512], vd)
                    if (c % 3) == 0:
                        # fused multiply+row-sum on DVE
                        nc.vector.scalar_tensor_tensor(
                            out=junk,
                            in0=u,
                            scalar=0.0,
                            in1=wout_sb[:, i, :],
                            op0=Alu.add,
                            op1=Alu.mult,
                            accum_out=sg[:, i:i + 1],
                        )
                    else:
                        # GPSIMD multiply, ACT row-sum via accumulate
                        nc.gpsimd.tensor_mul(junk, u, wout_sb[:, i, :])
                        nc.scalar.activation(
                            junk, junk, Act.Copy, bias=0.0, scale=1.0,
                            accum_out=sg[:, i:i + 1],
                        )

                # ---- per-group epilogue ----
                if not apply_gamma_beta:
                    sdg = gpool.tile([128, 4], f32, tag="sdg")
                    nc.scalar.activation(
                        sdg, mvg[:, :, 1], Act.Sqrt, bias=eps_sb, scale=1.0
                    )
                    rg = gpool.tile([128, 4], f32, tag="rg")
                    nc.vector.reciprocal(rg, sdg)
                    tmp = gpool.tile([128, 4], f32, tag="tmp")
                    nc.gpsimd.tensor_mul(tmp, sg, rg)
                    nc.gpsimd.tensor_add(
                        ocol[:, g * 4:(g + 1) * 4], tmp, bout_sb
                    )
                else:
                    nc.vector.tensor_add(
                        ocol[:, g * 4:(g + 1) * 4], sg, bout_sb
                    )

            # single 64KB output DMA; the [p, c] -> m = c*128 + p transpose
            # happens on the host (free)
            nc.sync.dma_start(out[:, :], ocol)

    nc.finalize()
    return nc


def _get_program(apply_gamma_beta: bool):
    key = bool(apply_gamma_beta)
    if key not in _programs:
        _programs[key] = _build_program(key)
    return _programs[key]


def kernel(**inputs) -> np.ndarray:
    x = np.asarray(inputs["x"], dtype=np.float32)
    W1 = np.asarray(inputs["W1"], dtype=np.float32)
    b1 = np.asarray(inputs["b1"], dtype=np.float32)
    gamma = np.asarray(inputs["gamma"], dtype=np.float32)
    beta = np.asarray(inputs["beta"], dtype=np.float32)
    Wout = np.asarray(inputs["Wout"], dtype=np.float32)
    bout = np.asarray(inputs["bout"], dtype=np.float32)

    assert x.shape == (B, T, D), x.shape

    fast = bool(np.all(gamma == 1.0) and np.all(beta == 0.0))
    nc = _get_program(apply_gamma_beta=not fast)

    # ---- host-side packing (free at device time) ----
    # W1 is [e, d]; device wants W1T chunks [p, dc, e] with d = dc*128 + p.
    # Fast path: negate W1T/b1 so PSUM holds -h1 and the bn_stats mean can be
    # used directly as the relu bias (relu(-P + mean) == relu(h1 - mu)).
    dt_mode = _matmul_dtype()
    if dt_mode == "bf16":
        import ml_dtypes

        mm_np = ml_dtypes.bfloat16
    else:
        mm_np = np.float32
    sgn = np.float32(-1.0 if fast else 1.0)
    w1t_np = np.ascontiguousarray(
        (sgn * W1.T).reshape(4, 128, D).transpose(1, 0, 2).astype(mm_np)
    )
    wout_np = np.ascontiguousarray(
        Wout.reshape(4, 128, D).transpose(1, 0, 2).astype(mm_np)
    )
    bout_np = np.ascontiguousarray(bout.reshape(4, 128).T)
    b1_np = np.ascontiguousarray(
        np.broadcast_to((sgn / 128.0) * b1, (128, D)).astype(mm_np)
    )

    shared = {"w1t": w1t_np, "b1": b1_np, "wout": wout_np, "bout": bout_np}
    if not fast:
        shared["gammab"] = np.ascontiguousarray(
            np.broadcast_to(gamma, (128, D))
        )
        shared["betab"] = np.ascontiguousarray(
            np.broadcast_to(beta, (128, D))
        )

    # drop CLS -> [256, 512, 512]; cast before the big permute so the
    # transpose moves half the bytes
    xs = np.asarray(x[:, : T - 1, :], dtype=mm_np)
    in_maps = []
    for c in range(NCORES):
        src = xs[c * BL:(c + 1) * BL].reshape(M, D)
        # [m, d] -> [p, g, dc, mm] with d = dc*128 + p, m = g*512 + mm
        xt_c = np.ascontiguousarray(
            src.reshape(NG, 512, 4, 128).transpose(3, 0, 2, 1)
        )
        in_maps.append({"xt": xt_c, **shared})

    from concourse import bass_utils

    trace = os.environ.get("KERNEL_TRACE") == "1"
    res = bass_utils.run_bass_kernel_spmd(
        nc, in_maps, core_ids=list(range(NCORES)), trace=trace
    )
    if trace:
        if res.exec_time_ns is not None:
            print(f"HW exec time: {res.exec_time_ns} ns")
            print(f"mean exec time: {res.mean_exec_time_ns} ns "
                  f"(slowest core {res.max_exec_time_core_id})")
        if res.instructions_and_trace is not None:
            print("trace:", res.instructions_and_trace[1])
        if res.profile_json is not None:
            print("profile json:", res.profile_json)

    out_full = np.empty((B, D), dtype=np.float32)
    for c, r in enumerate(res.results):
        # device out[p, tc] holds row m = tc*128 + p
        out_full[c * BL:(c + 1) * BL] = (
            np.ascontiguousarray(r["out"].T).reshape(BL, D)
        )
    return out_full



# revision 17
# speedup vs baseline: 1.3758x; 1.3758x over previous
"""Trainium2 Bass kernel for nn_ReconstructionHead (dense_mlp).

Computes, for x[B=256, T=513, D=512] (CLS token at t=512 dropped):
    h   = x[:, :512] @ W1.T + b1          # [256, 512, 512]
    h   = LayerNorm(h) * gamma + beta     # over last dim
    h   = relu(h)
    out[b, t] = h[b, t] @ Wout[t] + bout[t]   # [256, 512]

Sharding: data-parallel over batch across 8 NeuronCores (32 batches/core).
Weights are replicated. All input reshaping/transposition happens on the
host (numpy); the device sees clean strided layouts.

Per-core device program (fast path, gamma==1 / beta==0):
  - x is pre-transposed on host to xt[p, dc, m] with d = dc*128+p on SBUF
    partitions, m = b_local*512 + t  (16384 rows -> 128 tiles of 128 rows).
  - PE: per tile, a K=1 matmul seeds PSUM with b1 (broadcast), then 4
    accumulating 128x128x512 fp32 matmuls produce h1 = x @ W1.T + b1.
  - DVE bn_stats/bn_aggr give mean/var per row.
  - ACT computes u = relu(h1 - mu) straight out of PSUM (per-partition bias).
    Since rstd > 0, relu((h1-mu)*rstd) = rstd * relu(h1-mu), so the rstd
    multiply is deferred to the tiny per-column epilogue.
  - One scalar_tensor_tensor with accum_out computes s = sum_e u * Wout[t]
    (the per-column head), split between GPSIMD and DVE to balance engines.
  - Epilogue: out_col = s * rstd + bout, accumulated in a [128, 128] SBUF
    tile, PE-transposed once at the end for a contiguous output DMA.
"""

import os
import sys

import numpy as np

for _p in ("/root/.axon_site/_ro/trn_rl_repo", "/opt/trn_rl_repo"):
    if os.path.isdir(_p) and _p not in sys.path:
        sys.path.append(_p)

B = 256
T = 513
D = 512          # d_in == d_out
NCORES = 8
BL = B // NCORES          # 32 batches per core
M = BL * D                # 16384 rows per core
NT = M // 128             # 128 tiles per core
NG = NT // 4              # 32 groups (one group = 512 rows = one batch)
EPS = 1e-5

_programs = {}


def _matmul_dtype():
    """'bf16' (default), 'f32r', or 'f32' via KERNEL_DTYPE."""
    d = os.environ.get("KERNEL_DTYPE", "bf16")
    if os.environ.get("KERNEL_FP32_STRICT") == "1":
        return "f32"
    return d


def _build_program(apply_gamma_beta: bool):
    import concourse.bacc as bacc
    import concourse.tile as tile
    from concourse import mybir

    f32 = mybir.dt.float32
    dt_mode = _matmul_dtype()
    # bf16 matmuls stream at the same 1 cycle/row as float32r but get
    # FWL on LDWEIGHTS (4x faster weight loads that hide under the
    # previous matmul) and halve DMA + stage-2 vector-engine traffic.
    if dt_mode == "bf16":
        f32m = mybir.dt.bfloat16
    elif dt_mode == "f32r":
        f32m = mybir.dt.float32r
    else:
        f32m = f32
    # vector-side dtype for relu output / stage-2 operands
    vd = mybir.dt.bfloat16 if dt_mode == "bf16" else f32
    Alu = mybir.AluOpType
    Act = mybir.ActivationFunctionType

    nc = bacc.Bacc()
    xt = nc.dram_tensor("xt", [128, NG, 4, 512], f32m, kind="ExternalInput")
    w1t = nc.dram_tensor("w1t", [128, 4, D], f32m, kind="ExternalInput")
    # b1/128 replicated over 128 partitions: the bias seed is a K=128
    # matmul (ones.T @ b1rep) so its LDWEIGHTS pipelines exactly like the
    # main matmuls (a K=1 rank-1 seed costs two ~100ns boundary bubbles)
    b1 = nc.dram_tensor("b1", [128, D], f32m, kind="ExternalInput")
    wout = nc.dram_tensor("wout", [128, 4, D], vd, kind="ExternalInput")
    bout = nc.dram_tensor("bout", [128, 4], f32, kind="ExternalInput")
    if apply_gamma_beta:
        gammab = nc.dram_tensor("gammab", [128, D], f32, kind="ExternalInput")
        betab = nc.dram_tensor("betab", [128, D], f32, kind="ExternalInput")
    # out[p, c] = output for row m = c*128 + p (transposed on host)
    out = nc.dram_tensor("out", [128, 128], f32, kind="ExternalOutput")

    with tile.TileContext(nc) as tc:
        with (
            tc.tile_pool(name="singles", bufs=1) as singles,
            tc.tile_pool(name="xg", bufs=4) as xpool,
            tc.tile_pool(name="u", bufs=8) as upool,
            tc.tile_pool(name="junk", bufs=4) as jpool,
            tc.tile_pool(name="stats", bufs=12) as spool,
            tc.tile_pool(name="grp", bufs=4) as gpool,
            tc.tile_pool(name="psum", bufs=8, space="PSUM") as psum_pool,
        ):
            # ---- static tiles ----
            # ordered so the first matmul's dependencies land first
            b1_sb = singles.tile([128, D], f32m)
            nc.sync.dma_start(b1_sb, b1[:, :])
            w1t_sb = singles.tile([128, 4, D], f32m)
            nc.sync.dma_start(w1t_sb, w1t[:, :, :])

            def load_group(g):
                xg = xpool.tile([128, 4, 512], f32m, tag="xg")
                nc.sync.dma_start(xg, xt[:, g, :, :])
                return xg

            xg_next = load_group(0)

            wout_sb = singles.tile([128, 4, D], vd)
            nc.sync.dma_start(wout_sb, wout[:, :, :])
            bout_sb = singles.tile([128, 4], f32)
            nc.sync.dma_start(bout_sb, bout[:, :])
            if f32m is mybir.dt.float32r:
                ones_sb = singles.tile([128, 128], f32)
                nc.vector.memset(ones_sb, 1.0)
                ones_mm = ones_sb.bitcast(f32m)
            else:
                ones_mm = singles.tile([128, 128], f32m)
                nc.vector.memset(ones_mm, 1.0)
            eps_sb = singles.tile([128, 1], f32)
            nc.vector.memset(eps_sb, EPS)
            ocol = singles.tile([128, 128], f32)  # per-tile output columns
            if apply_gamma_beta:
                gamma_sb = singles.tile([128, D], f32)
                nc.sync.dma_start(gamma_sb, gammab[:, :])
                beta_sb = singles.tile([128, D], f32)
                nc.sync.dma_start(beta_sb, betab[:, :])

            for g in range(NG):
                xg = xg_next
                if g + 1 < NG:
                    xg_next = load_group(g + 1)

                mvg = gpool.tile([128, 4, 2], f32)   # (mean, var) per tile
                sg = gpool.tile([128, 4], f32)       # stage-2 raw sums

                for i in range(4):
                    c = g * 4 + i
                    P = psum_pool.tile([128, 512], f32)
                    # seed PSUM with b1 (rank-1 matmul), then accumulate x@W1T
                    nc.tensor.matmul(P, ones_mm, b1_sb, start=True, stop=False)
                    for dc in range(4):
                        nc.tensor.matmul(
                            P,
                            xg[:, dc, i * 128:(i + 1) * 128],
                            w1t_sb[:, dc, :],
                            start=False,
                            stop=(dc == 3),
                        )

                    st6 = spool.tile([128, 6], f32)
                    nc.vector.bn_stats(st6, P)
                    nc.vector.bn_aggr(mvg[:, i, :], st6)

                    if not apply_gamma_beta:
                        # Host negated W1T/b1, so P holds -h1 and bn_stats'
                        # mean is -mu: u = relu(-1*P + mean) = relu(h1 - mu).
                        # rstd multiplication is deferred to the epilogue.
                        u = upool.tile([128, 512], vd)
                        nc.scalar.activation(
                            u, P, Act.Relu, bias=mvg[:, i, 0:1], scale=-1.0
                        )
                    else:
                        # full path: n = (h1 - mu) * rstd ; z = n*gamma + beta
                        sd = spool.tile([128, 1], f32, tag="sd")
                        nc.scalar.activation(
                            sd, mvg[:, i, 1:2], Act.Sqrt, bias=eps_sb, scale=1.0
                        )
                        rr = spool.tile([128, 1], f32, tag="rr")
                        nc.vector.reciprocal(rr, sd)
                        n_sb = upool.tile([128, 512], f32, tag="n")
                        nc.vector.tensor_scalar(
                            out=n_sb,
                            in0=P,
                            scalar1=mvg[:, i, 0:1],
                            scalar2=rr,
                            op0=Alu.subtract,
                            op1=Alu.mult,
                        )
                        v_sb = upool.tile([128, 512], f32, tag="v")
                        nc.gpsimd.tensor_mul(v_sb, n_sb, gamma_sb)
                        z_sb = upool.tile([128, 512], f32, tag="z")
                        nc.vector.tensor_add(z_sb, v_sb, beta_sb)
                        u = upool.tile([128, 512], vd)
                        nc.scalar.activation(u, z_sb, Act.Relu)

                    # stage-2: s = sum_e u * Wout[t-block i]
                    junk = jpool.tile([128, 512], vd)
                    if (c % 2) == 0:
                        # fused multiply+row-sum on DVE
                        nc.vector.scalar_tensor_tensor(
                            out=junk,
                            in0=u,
                            scalar=0.0,
                            in1=wout_sb[:, i, :],
                            op0=Alu.add,
                            op1=Alu.mult,
                            accum_out=sg[:, i:i + 1],
                        )
                    else:
                        # GPSIMD multiply, ACT row-sum via accumulate
                        nc.gpsimd.tensor_mul(junk, u, wout_sb[:, i, :])
                        nc.scalar.activation(
                            junk, junk, Act.Copy, bias=0.0, scale=1.0,
                            accum_out=sg[:, i:i + 1],
                        )

                # ---- per-group epilogue ----
                if not apply_gamma_beta:
                    sdg = gpool.tile([128, 4], f32, tag="sdg")
                    nc.scalar.activation(
                        sdg, mvg[:, :, 1], Act.Sqrt, bias=eps_sb, scale=1.0
                    )
                    rg = gpool.tile([128, 4], f32, tag="rg")
                    nc.vector.reciprocal(rg, sdg)
                    tmp = gpool.tile([128, 4], f32, tag="tmp")
                    nc.gpsimd.tensor_mul(tmp, sg, rg)
                    nc.gpsimd.tensor_add(
                        ocol[:, g * 4:(g + 1) * 4], tmp, bout_sb
                    )
                else:
                    nc.vector.tensor_add(
                        ocol[:, g * 4:(g + 1) * 4], sg, bout_sb
                    )

            # single 64KB output DMA; the [p, c] -> m = c*128 + p transpose
            # happens on the host (free)
            nc.sync.dma_start(out[:, :], ocol)

    nc.finalize()
    return nc


def _get_program(apply_gamma_beta: bool):
    key = bool(apply_gamma_beta)
    if key not in _programs:
        _programs[key] = _build_program(key)
    return _programs[key]


def kernel(**inputs) -> np.ndarray:
    x = np.asarray(inputs["x"], dtype=np.float32)
    W1 = np.asarray(inputs["W1"], dtype=np.float32)
    b1 = np.asarray(inputs["b1"], dtype=np.float32)
    gamma = np.asarray(inputs["gamma"], dtype=np.float32)
    beta = np.asarray(inputs["beta"], dtype=np.float32)
    Wout = np.asarray(inputs["Wout"], dtype=np.float32)
    bout = np.asarray(inputs["bout"], dtype=np.float32)

    assert x.shape == (B, T, D), x.shape

    fast = bool(np.all(gamma == 1.0) and np.all(beta == 0.0))
    nc = _get_program(apply_gamma_beta=not fast)

    # ---- host-side packing (free at device time) ----
    # W1 is [e, d]; device wants W1T chunks [p, dc, e] with d = dc*128 + p.
    # Fast path: negate W1T/b1 so PSUM holds -h1 and the bn_stats mean can be
    # used directly as the relu bias (relu(-P + mean) == relu(h1 - mu)).
    dt_mode = _matmul_dtype()
    if dt_mode == "bf16":
        import ml_dtypes

        mm_np = ml_dtypes.bfloat16
    else:
        mm_np = np.float32
    sgn = np.float32(-1.0 if fast else 1.0)
    w1t_np = np.ascontiguousarray(
        (sgn * W1.T).reshape(4, 128, D).transpose(1, 0, 2).astype(mm_np)
    )
    wout_np = np.ascontiguousarray(
        Wout.reshape(4, 128, D).transpose(1, 0, 2).astype(mm_np)
    )
    bout_np = np.ascontiguousarray(bout.reshape(4, 128).T)
    b1_np = np.ascontiguousarray(
        np.broadcast_to((sgn / 128.0) * b1, (128, D)).astype(mm_np)
    )

    shared = {"w1t": w1t_np, "b1": b1_np, "wout": wout_np, "bout": bout_np}
    if not fast:
        shared["gammab"] = np.ascontiguousarray(
            np.broadcast_to(gamma, (128, D))
        )
        shared["betab"] = np.ascontiguousarray(
            np.broadcast_to(beta, (128, D))
        )

    # drop CLS -> [256, 512, 512]; cast before the big permute so the
    # transpose moves half the bytes
    xs = np.asarray(x[:, : T - 1, :], dtype=mm_np)
    in_maps = []
    for c in range(NCORES):
        src = xs[c * BL:(c + 1) * BL].reshape(M, D)
        # [m, d] -> [p, g, dc, mm] with d = dc*128 + p, m = g*512 + mm
        xt_c = np.ascontiguousarray(
            src.reshape(NG, 512, 4, 128).transpose(3, 0, 2, 1)
        )
        in_maps.append({"xt": xt_c, **shared})

    from concourse import bass_utils

    trace = os.environ.get("KERNEL_TRACE") == "1"
    res = bass_utils.run_bass_kernel_spmd(
        nc, in_maps, core_ids=list(range(NCORES)), trace=trace
    )
    if trace:
        if res.exec_time_ns is not None:
            print(f"HW exec time: {res.exec_time_ns} ns")
            print(f"mean exec time: {res.mean_exec_time_ns} ns "
                  f"(slowest core {res.max_exec_time_core_id})")
        if res.instructions_and_trace is not None:
            print("trace:", res.instructions_and_trace[1])
        if res.profile_json is not None:
            print("profile json:", res.profile_json)

    out_full = np.empty((B, D), dtype=np.float32)
    for c, r in enumerate(res.results):
        # device out[p, tc] holds row m = tc*128 + p
        out_full[c * BL:(c + 1) * BL] = (
            np.ascontiguousarray(r["out"].T).reshape(BL, D)
        )
    return out_full

